# revision 41
# baseline (speedup 1.0000x reference)
"""ComputeAlignmentError kernel for 8 TRN2 NeuronCores.

Math: for each batch b, pairwise alignment error
    err[i,j] = || Ep_j (pc_i - bp_j) - Et_j (tc_i - bt_j) + eps ||_2
where Ep/Et are orthonormal frame bases built from pred/true frames and
bp/bt are the frame origins.  Because Ep/Et are rotations, err^2 collapses
into a rank-18 bilinear form  err^2[i,j] = Y[i] . Z[j]  with
    Y[i] = [1, |pc|^2, |tc|^2, -2pc, -2tc, -2vec(pc tc^T)]   (18)
    Z[j] = [z0, 1, 1, (bp - R bt), (bt - R^T bp), vec(R)]
    R_j = Ep_j^T Et_j,  z0 = bp.(bp - 2 R bt) + bt.bt
(the reference's eps=1e-8 terms shift err^2 by ~1e-7 -- far below fp32
noise -- and are dropped).  Mask folds in: Y *= mask_i, Z *= mask_j.

Frame basis without normalizing w1/w2 first (saves a normalize round):
    u = a - b, v = c - b
    e1 = normalize(|v| u + |u| v), e2 = normalize(|u| v - |v| u),
    e3 = normalize(u x v)        (e1 _|_ e2 exactly, so |e1 x e2| = 1)

Each core handles one (batch, 512-row i-slice).  Frames are laid out
chunk-major (chunk, set) so a leading slice of chunks for BOTH sets is
one contiguous group range.  Critical-path ordering:
  - one merged input DMA; dummy sqrt pins the ACT sqrt table at t=0
  - stage A for chunks 0-3 first (Vector), then chunks 4-15
  - stage B: chunks 0-3, 4-7 on Vector; 8-15 on Pool; Y on Pool
  - PE transposes; PSUM->SBUF copies spread over ACT/Vector/Pool
  - output column blocks ordered j-low-first across all i-tiles so the
    out-DMA stream starts as early as possible; K=18 float32r matmuls
    (4x PE rate, no row packing); sqrt+bias on ACT guards f32r round-off
"""

import sys

import numpy as np

sys.path.insert(0, "/opt/trn_rl_repo")

from contextlib import ExitStack

import concourse.bacc as bacc
import concourse.bass as bass
import concourse.tile as tile
from concourse import mybir
from concourse.bass_utils import run_bass_kernel_spmd
from concourse.masks import make_identity

F32 = mybir.dt.float32
F32R = mybir.dt.float32r
AF = mybir.ActivationFunctionType
OP = mybir.AluOpType

B, N = 2, 2048
NCORES = 8
ISLICE = N * B // NCORES  # 512 rows of i per core
NITILE = ISLICE // 128  # 4 i-tiles per core
NJCH = N // 128  # 16 j-chunks
NF = 18  # feature count K
FPAD = 32  # feature slot padding (PSUM partition alignment after transpose)
G = 2 * NJCH  # 32 (chunk, set) groups

USE_F32R = True
SQRT_BIAS = 1e-2  # guards f32r round-off pushing err^2 slightly negative

NIN = NJCH * 18 + NITILE * 6 + NJCH + NITILE  # 332 floats per partition


def _build(nc_holder=[], amp=None):
    if nc_holder and amp is None:
        return nc_holder[0]
    nc = bacc.Bacc(
        "TRN2",
        target_bir_lowering=False,
        debug=False,
        enable_asserts=True,
        num_devices=NCORES,
    )
    inp_in = nc.dram_tensor("inp", [128, NIN], F32, kind="ExternalInput").ap()
    out_dram = nc.dram_tensor("out", [ISLICE, N], F32, kind="ExternalOutput").ap()

    with tile.TileContext(nc) as tc, ExitStack() as ctx:
        if amp is None:
            _kernel_body(ctx, tc, out_dram, inp_in)
        else:
            with tc.For_i(0, amp, 1):
                with ExitStack() as ictx:
                    _kernel_body(ictx, tc, out_dram, inp_in)

    nc.compile()
    if amp is None:
        nc_holder.append(nc)
    return nc


U32 = mybir.dt.uint32
RSQRT_MAGIC = 0x5F3759DF


def _rsqrt_dve(nc, sb, eng, y, x, tag, iters=2):
    """y = rsqrt(x) computed entirely on one vector-ish engine: magic-
    constant seed (shift/xor/add on the bit pattern) + Newton iterations.
    Rel err <= ~5e-6 after 2 iterations -- plenty for basis normalization.
    Avoids the ACT sqrt round-trip on the critical path."""
    shape = list(y.shape)
    u = sb.tile(shape, U32, tag=f"rsu{tag}")
    eng.tensor_scalar(u[:], x.bitcast(U32), 1, None, OP.logical_shift_right)
    # y0 bits = MAGIC - (bits >> 1)  ==  (u xor 0xFFFFFFFF) + MAGIC + 1
    eng.tensor_scalar(
        y.bitcast(U32), u[:], 0xFFFFFFFF, RSQRT_MAGIC + 1, OP.bitwise_xor, OP.add
    )
    a = sb.tile(shape, F32, tag=f"rsa{tag}")
    for _ in range(iters):
        eng.tensor_mul(a[:], y, y)
        eng.tensor_mul(a[:], a[:], x)
        eng.tensor_scalar(a[:], a[:], -0.5, 1.5, OP.mult, OP.add)
        eng.tensor_mul(y, y, a[:])


def _frame_bases(nc, sb, eng, Fg, Est, g0, g1, fast=False):
    """Stage A for group range [g0, g1) on one engine; writes Est[:, g0:g1].
    fast=True keeps everything on `eng` via DVE-style rsqrt (no ACT hops);
    otherwise the two sqrt calls go to ACT and reciprocal runs on Vector."""
    P, C = 128, g1 - g0
    is_v = eng is nc.vector
    tg = f"{g0}"
    Fs = Fg[:, g0:g1]  # [p, g, pt, xyz]

    def red3(out, in_, tagbase):
        if is_v:
            nc.vector.reduce_sum(out.unsqueeze(3), in_, axis=mybir.AxisListType.X)
        else:
            t = sb.tile(list(out.shape), F32, tag=f"{tagbase}{tg}")
            eng.tensor_add(t[:], in_[:, :, :, 0], in_[:, :, :, 1])
            eng.tensor_add(out, t[:], in_[:, :, :, 2])

    w12 = sb.tile([P, C, 2, 3], F32, tag=f"w12{tg}")
    eng.tensor_sub(
        w12[:], Fs[:, :, 0::2, :], Fs[:, :, 1, :].unsqueeze(2).broadcast_to((P, C, 2, 3))
    )
    sqw = sb.tile([P, C, 2, 3], F32, tag=f"sqw{tg}")
    eng.tensor_mul(sqw[:], w12[:], w12[:])
    n2 = sb.tile([P, C, 2], F32, tag=f"n2{tg}")
    red3(n2[:], sqw[:], "rda")
    nrm = sb.tile([P, C, 2], F32, tag=f"nrm{tg}")
    if fast:
        ry = sb.tile([P, C, 2], F32, tag=f"ry{tg}")
        _rsqrt_dve(nc, sb, eng, ry[:], n2[:], f"a{tg}")
        eng.tensor_mul(nrm[:], n2[:], ry[:])  # |u| = n2 * rsqrt(n2)
    else:
        nc.scalar.sqrt(nrm[:], n2[:])

    # cross r = u x v via duplicated-copy rotation trick (no nrm dep:
    # scheduled while ACT does the sqrt)
    cbuf = sb.tile([P, C, 2, 6], F32, tag=f"cbuf{tg}")
    eng.tensor_copy(
        cbuf[:].rearrange("p g w (d x) -> p (g w) d x", d=2),
        w12[:]
        .rearrange("p g w x -> p (g w) x")
        .unsqueeze(2)
        .broadcast_to((P, C * 2, 2, 3)),
    )
    pqr = sb.tile([P, C, 3, 3], F32, tag=f"pqr{tg}")
    mtmp = sb.tile([P, C, 2, 3], F32, tag=f"mtmp{tg}")
    eng.tensor_mul(mtmp[:, :, 0, :], cbuf[:, :, 0, 1:4], cbuf[:, :, 1, 2:5])
    eng.tensor_mul(mtmp[:, :, 1, :], cbuf[:, :, 0, 2:5], cbuf[:, :, 1, 1:4])
    eng.tensor_sub(pqr[:, :, 2, :], mtmp[:, :, 0, :], mtmp[:, :, 1, :])

    tt = sb.tile([P, C, 2, 3], F32, tag=f"tt{tg}")
    eng.tensor_mul(
        tt[:, :, 0, :], w12[:, :, 0, :], nrm[:, :, 1].unsqueeze(2).broadcast_to((P, C, 3))
    )
    eng.tensor_mul(
        tt[:, :, 1, :], w12[:, :, 1, :], nrm[:, :, 0].unsqueeze(2).broadcast_to((P, C, 3))
    )
    eng.tensor_add(pqr[:, :, 0, :], tt[:, :, 0, :], tt[:, :, 1, :])
    eng.tensor_sub(pqr[:, :, 1, :], tt[:, :, 1, :], tt[:, :, 0, :])

    sq2 = sb.tile([P, C, 3, 3], F32, tag=f"sq2{tg}")
    eng.tensor_mul(
        sq2[:].rearrange("p g e x -> p (g e x)"),
        pqr[:].rearrange("p g e x -> p (g e x)"),
        pqr[:].rearrange("p g e x -> p (g e x)"),
    )
    n2b = sb.tile([P, C, 3], F32, tag=f"n2b{tg}")
    red3(n2b[:], sq2[:], "rdb")
    rinv = sb.tile([P, C, 3], F32, tag=f"rinv{tg}")
    if fast:
        _rsqrt_dve(nc, sb, eng, rinv[:], n2b[:], f"b{tg}")
    else:
        nrmb = sb.tile([P, C, 3], F32, tag=f"nrmb{tg}")
        nc.scalar.sqrt(nrmb[:], n2b[:])
        nc.vector.reciprocal(rinv[:], nrmb[:])
    eng.tensor_mul(
        Est[:, g0:g1], pqr[:], rinv[:].unsqueeze(3).broadcast_to((P, C, 3, 3))
    )


def _z_range(nc, sb, eng, Zb, Est, Ft, Mj, c0, c1):
    """Stage B (Z features) for j-chunks [c0, c1) on one engine."""
    P, C = 128, c1 - c0
    is_v = eng is nc.vector
    ch = slice(c0, c1)
    tg = f"{c0}"

    Estv = Est[:].rearrange("p (c s) e x -> p c s e x", s=2)
    Ep = Estv[:, ch, 0]  # [p, c, k, xyz]
    Et = Estv[:, ch, 1]
    bp = Ft[:, ch, 0, 1, :]  # [p, c, xyz]
    bt = Ft[:, ch, 1, 1, :]
    bpbt = Ft[:, ch, :, 1, :]  # [p, c, set, xyz]
    Zh = Zb[:, ch]
    Rz = Zh[:, :, 9:18].rearrange("p c (a b) -> p c a b", a=3)

    def red3(out, in_, tagbase):
        if is_v:
            nc.vector.reduce_sum(out.unsqueeze(3), in_, axis=mybir.AxisListType.X)
        else:
            t = sb.tile(list(out.shape), F32, tag=f"{tagbase}{tg}")
            eng.tensor_add(t[:], in_[:, :, :, 0], in_[:, :, :, 1])
            eng.tensor_add(out, t[:], in_[:, :, :, 2])

    # R[c,a,b] = sum_k Ep[c,k,a] * Et[c,k,b]  -> Zb slots 9..17
    prodR = sb.tile([P, C, 9, 3], F32, tag=f"prodR{tg}")
    for a in range(3):
        eng.tensor_mul(
            prodR[:, :, 3 * a : 3 * a + 3, :],
            Ep[:, :, :, a].unsqueeze(2).broadcast_to((P, C, 3, 3)),
            Et.transpose([0, 1, 3, 2]),
        )
    red3(Zh[:, :, 9:18], prodR[:], "rdr")

    # Rbt[c,a] = sum_b R[c,a,b] bt[c,b] ; Rtbp[c,b] = sum_a R[c,a,b] bp[c,a]
    prodv = sb.tile([P, C, 6, 3], F32, tag=f"prodv{tg}")
    eng.tensor_mul(prodv[:, :, 0:3, :], Rz, bt.unsqueeze(2).broadcast_to((P, C, 3, 3)))
    eng.tensor_mul(
        prodv[:, :, 3:6, :],
        Rz.transpose([0, 1, 3, 2]),
        bp.unsqueeze(2).broadcast_to((P, C, 3, 3)),
    )
    Rv = sb.tile([P, C, 2, 3], F32, tag=f"Rv{tg}")
    red3(Rv[:].rearrange("p c v x -> p c (v x)"), prodv[:], "rdv")

    # Zb[3:9] = (bp, bt) - (Rbt, Rtbp)
    eng.tensor_sub(
        Zh[:, :, 3:9].rearrange("p c (s x) -> p c s x", s=2), bpbt, Rv[:]
    )
    # z0 = bp.(bp - 2 Rbt) + bt.bt ; (bp - 2 Rbt) = Zb[3:6] - Rbt
    ph = sb.tile([P, C, 2, 3], F32, tag=f"ph{tg}")
    eng.tensor_sub(ph[:, :, 0, :], Zh[:, :, 3:6], Rv[:, :, 0, :])
    eng.tensor_copy(ph[:, :, 1, :], bt)
    prodH = sb.tile([P, C, 2, 3], F32, tag=f"prodH{tg}")
    eng.tensor_mul(prodH[:], bpbt, ph[:])
    if is_v:
        nc.vector.reduce_sum(
            Zh[:, :, 0:1],
            prodH[:].rearrange("p c s x -> p c (s x)"),
            axis=mybir.AxisListType.X,
        )
    else:
        t = sb.tile([P, C, 3], F32, tag=f"rdz{tg}")
        eng.tensor_add(t[:], prodH[:, :, 0, :], prodH[:, :, 1, :])
        t2 = sb.tile([P, C, 1], F32, tag=f"rdz2{tg}")
        eng.tensor_add(t2[:], t[:, :, 0:1], t[:, :, 1:2])
        eng.tensor_add(Zh[:, :, 0:1], t2[:], t[:, :, 2:3])

    eng.memset(Zh[:, :, 1:3], 1.0)
    eng.tensor_mul(
        Zh[:, :, 0:NF],
        Zh[:, :, 0:NF],
        Mj[:, ch].unsqueeze(2).broadcast_to((P, C, NF)),
    )


def _kernel_body(ctx, tc, out_dram, inp_in):
    nc = tc.nc
    P = 128
    sb = ctx.enter_context(tc.tile_pool(name="sb", bufs=1))
    outp = ctx.enter_context(tc.tile_pool(name="outp", bufs=8))
    pst = ctx.enter_context(tc.tile_pool(name="pst", bufs=2, space="PSUM"))
    ps5 = ctx.enter_context(tc.tile_pool(name="ps5", bufs=4, space="PSUM"))

    # ---- one merged input DMA --------------------------------------------
    inp = sb.tile([P, NIN], F32, tag="inp")
    nc.sync.dma_start(out=inp[:], in_=inp_in[:])
    o0 = NJCH * 18
    o1 = o0 + NITILE * 6
    o2 = o1 + NJCH
    # frames chunk-major: [p, chunk, set, pt, xyz]
    Ft = inp[:, 0:o0].rearrange("p (c s t x) -> p c s t x", c=NJCH, s=2, t=3)
    Ct = inp[:, o0:o1].rearrange("p (c s x) -> p c s x", c=NITILE, s=2)
    Mj = inp[:, o1:o2]  # [p, NJCH]
    Mi = inp[:, o2:NIN]  # [p, NITILE]
    Fg = Ft[:].rearrange("p c s t x -> p (c s) t x")  # [p, g, pt, xyz]

    ident = sb.tile([P, P], F32, tag="ident")
    make_identity(nc, ident[:])
    bias_t = sb.tile([P, 1], F32, tag="bias")
    nc.gpsimd.memset(bias_t[:], SQRT_BIAS)
    # dependency-free sqrt pins the sqrt_and_others ACT table at t=0
    dumm = sb.tile([P, 1], F32, tag="dumm")
    nc.scalar.sqrt(dumm[:], bias_t[:])

    # ---- Y features (Pool; only needs coords) -----------------------------
    Yb = sb.tile([P, NITILE, FPAD], F32, tag="Yb")
    nc.gpsimd.tensor_scalar_mul(
        Yb[:, :, 3:9], Ct[:].rearrange("p c s x -> p c (s x)"), -2.0
    )
    sqc = sb.tile([P, NITILE, 2, 3], F32, tag="sqc")
    nc.gpsimd.tensor_mul(
        sqc[:].rearrange("p c s x -> p (c s x)"),
        Ct[:].rearrange("p c s x -> p (c s x)"),
        Ct[:].rearrange("p c s x -> p (c s x)"),
    )
    # |pc|^2, |tc|^2 via strided adds (Pool can't free-reduce)
    yt1 = sb.tile([P, NITILE, 2], F32, tag="yt1")
    nc.gpsimd.tensor_add(yt1[:], sqc[:, :, :, 0], sqc[:, :, :, 1])
    nc.gpsimd.tensor_add(Yb[:, :, 1:3], yt1[:], sqc[:, :, :, 2])
    # Yb[9:18] = (-2 pc) (x) tc
    nc.gpsimd.tensor_mul(
        Yb[:, :, 9:18].rearrange("p c (a b) -> p c a b", a=3),
        Yb[:, :, 3:6].unsqueeze(3).broadcast_to((P, NITILE, 3, 3)),
        Ct[:, :, 1, :].unsqueeze(2).broadcast_to((P, NITILE, 3, 3)),
    )
    nc.gpsimd.memset(Yb[:, :, 0:1], 1.0)
    nc.gpsimd.tensor_mul(
        Yb[:, :, 0:NF],
        Yb[:, :, 0:NF],
        Mi[:].unsqueeze(2).broadcast_to((P, NITILE, NF)),
    )

    # ---- stage A + B + transposes + main loop, emitted in pipeline order --
    mm_dt = F32R if USE_F32R else F32
    Est = sb.tile([P, G, 3, 3], F32, tag="Est")
    Zb = sb.tile([P, NJCH, FPAD], F32, tag="Zb")
    YT = sb.tile([NF, NITILE * P], mm_dt, tag="YT")
    ZT = sb.tile([NF, N], mm_dt, tag="ZT")

    def z_transpose(g, engs):
        """Transpose+copy Zb chunks 4g..4g+3 into ZT."""
        ptz = pst.tile([P, P], F32, tag="tp")
        nc.tensor.transpose(
            ptz[:], Zb[:, 4 * g : 4 * g + 4, :].rearrange("p c f -> p (c f)"), ident[:]
        )
        for cl in range(4):
            c = 4 * g + cl
            dst = ZT[0:NF, c * P : (c + 1) * P]
            src = ptz[cl * FPAD : cl * FPAD + NF, :]
            eng = engs[cl]
            (eng.copy if eng is nc.scalar else eng.tensor_copy)(dst, src)

    def block(it, col0):
        pm = ps5.tile([P, 512], F32, tag="mm")
        nc.tensor.matmul(
            pm[:],
            YT[0:NF, it * P : (it + 1) * P],
            ZT[0:NF, col0 : col0 + 512],
            start=True,
            stop=True,
        )
        ot = outp.tile([P, 512], F32, tag="ot")
        nc.scalar.activation(ot[:], pm[:], AF.Sqrt, bias=bias_t[:])
        nc.sync.dma_start(
            out=out_dram[it * P : (it + 1) * P, col0 : col0 + 512], in_=ot[:]
        )

    # fast path: chunks 0-3 end-to-end feeds the first four output blocks
    with tc.high_priority():
        _frame_bases(nc, sb, nc.vector, Fg, Est, 0, 8)  # chunks 0-3
        _z_range(nc, sb, nc.vector, Zb[:], Est, Ft, Mj, 0, 4)

    _frame_bases(nc, sb, nc.vector, Fg, Est, 8, G)  # chunks 4-15

    z_transpose(0, [nc.vector] * 4)
    # Y transpose + copies AFTER the g0 transpose: the Y copies land on
    # ACT behind the fast-path sqrts instead of ahead of them (head-of-
    # line), and are still ready before the first matmul needs YT
    pt = pst.tile([P, P], F32, tag="tp")
    nc.tensor.transpose(pt[:], Yb[:].rearrange("p c f -> p (c f)"), ident[:])
    for c in range(NITILE):
        nc.scalar.copy(YT[0:NF, c * P : (c + 1) * P], pt[c * FPAD : c * FPAD + NF, :])
    for it in range(NITILE):
        block(it, 0)

    _z_range(nc, sb, nc.vector, Zb[:], Est, Ft, Mj, 4, 8)
    z_transpose(1, [nc.vector] * 4)
    for it in range(NITILE):
        block(it, 512)

    _z_range(nc, sb, nc.gpsimd, Zb[:], Est, Ft, Mj, 8, 16)
    z_transpose(2, [nc.vector] * 4)
    for it in range(NITILE):
        block(it, 1024)
    z_transpose(3, [nc.vector] * 4)
    for it in range(NITILE):
        block(it, 1536)


def _shard_inputs(pred_coords, true_coords, pred_frames, true_frames, mask):
    """Host-side reformat into per-core DMA-friendly layouts."""
    pc = np.asarray(pred_coords, np.float32)
    tc = np.asarray(true_coords, np.float32)
    pf = np.asarray(pred_frames, np.float32)
    tf = np.asarray(true_frames, np.float32)
    mk = np.asarray(mask).astype(np.float32)

    in_maps = []
    for core in range(NCORES):
        b = core // (NCORES // B)
        i0 = (core % (NCORES // B)) * ISLICE
        # frames [128, chunk, set, pt, xyz] ; input frames are [n, xyz, pt]
        fr = np.stack([pf[b], tf[b]], axis=0)  # [2, n, 3xyz, 3pt]
        fr = fr.transpose(0, 1, 3, 2)  # [2, n, pt, xyz]
        fr = fr.reshape(2, NJCH, 128, 3, 3).transpose(2, 1, 0, 3, 4)
        frames = fr.reshape(128, -1)
        # coords [128, chunk, set, xyz]
        co = np.stack([pc[b, i0 : i0 + ISLICE], tc[b, i0 : i0 + ISLICE]], axis=1)
        co = co.reshape(NITILE, 128, 2, 3).transpose(1, 0, 2, 3)
        coords = co.reshape(128, -1)
        maskj = mk[b].reshape(NJCH, 128).T
        maski = mk[b, i0 : i0 + ISLICE].reshape(NITILE, 128).T
        inp = np.ascontiguousarray(
            np.concatenate([frames, coords, maskj, maski], axis=1)
        )
        in_maps.append({"inp": inp})
    return in_maps


def kernel(pred_coords, true_coords, pred_frames, true_frames, mask, _res=[]):
    nc = _build()
    in_maps = _shard_inputs(pred_coords, true_coords, pred_frames, true_frames, mask)
    res = run_bass_kernel_spmd(nc, in_maps, list(range(NCORES)))
    _res.clear()
    _res.append(res)
    out = np.empty((B, N, N), np.float32)
    for core in range(NCORES):
        b = core // (NCORES // B)
        i0 = (core % (NCORES // B)) * ISLICE
        out[b, i0 : i0 + ISLICE, :] = res.results[core]["out"]
    return out


if __name__ == "__main__":
    rng = np.random.default_rng(0)
    ins = {
        "pred_coords": rng.standard_normal((B, N, 3)).astype(np.float32),
        "true_coords": rng.standard_normal((B, N, 3)).astype(np.float32),
        "pred_frames": rng.standard_normal((B, N, 3, 3)).astype(np.float32),
        "true_frames": rng.standard_normal((B, N, 3, 3)).astype(np.float32),
        "mask": np.ones((B, N), bool),
    }
    out = kernel(**ins)
    print("out", out.shape, out.dtype, float(np.abs(out).max()))


# revision 43
# speedup vs baseline: 1.7568x; 1.7568x over previous
"""ComputeAlignmentError kernel for 8 TRN2 NeuronCores.

Math: for each batch b, pairwise alignment error
    err[i,j] = || Ep_j (pc_i - bp_j) - Et_j (tc_i - bt_j) + eps ||_2
where Ep/Et are orthonormal frame bases built from pred/true frames and
bp/bt are the frame origins.  Because Ep/Et are rotations, err^2 collapses
into a rank-18 bilinear form  err^2[i,j] = Y[i] . Z[j]  with
    Y[i] = [1, |pc|^2, |tc|^2, -2pc, -2tc, -2vec(pc tc^T)]   (18)
    Z[j] = [z0, 1, 1, (bp - R bt), (bt - R^T bp), vec(R)]
    R_j = Ep_j^T Et_j,  z0 = bp.(bp - 2 R bt) + bt.bt
(the reference's eps=1e-8 terms shift err^2 by ~1e-7 -- far below fp32
noise -- and are dropped).  Mask folds in: Y *= mask_i, Z *= mask_j.

Frame basis without normalizing w1/w2 first (saves a normalize round):
    u = a - b, v = c - b
    e1 = normalize(|v| u + |u| v), e2 = normalize(|u| v - |v| u),
    e3 = normalize(u x v)        (e1 _|_ e2 exactly, so |e1 x e2| = 1)

Each core handles one (batch, 512-row i-slice).  Frames are laid out
chunk-major (chunk, set) so a leading slice of chunks for BOTH sets is
one contiguous group range.  Critical-path ordering:
  - one merged input DMA; dummy sqrt pins the ACT sqrt table at t=0
  - stage A for chunks 0-3 first (Vector), then chunks 4-15
  - stage B: chunks 0-3, 4-7 on Vector; 8-15 on Pool; Y on Pool
  - PE transposes; PSUM->SBUF copies spread over ACT/Vector/Pool
  - output column blocks ordered j-low-first across all i-tiles so the
    out-DMA stream starts as early as possible; K=18 float32r matmuls
    (4x PE rate, no row packing); sqrt+bias on ACT guards f32r round-off
"""

import sys

import numpy as np

sys.path.insert(0, "/opt/trn_rl_repo")

from contextlib import ExitStack

import concourse.bacc as bacc
import concourse.bass as bass
import concourse.tile as tile
from concourse import mybir
from concourse.bass_utils import run_bass_kernel_spmd
from concourse.masks import make_identity

F32 = mybir.dt.float32
F32R = mybir.dt.float32r
AF = mybir.ActivationFunctionType
OP = mybir.AluOpType

B, N = 2, 2048
NCORES = 8
ISLICE = N * B // NCORES  # 512 rows of i per core
NITILE = ISLICE // 128  # 4 i-tiles per core
NJCH = N // 128  # 16 j-chunks
NF = 18  # feature count K
FPAD = 32  # feature slot padding (PSUM partition alignment after transpose)
G = 2 * NJCH  # 32 (chunk, set) groups

USE_F32R = True
SQRT_BIAS = 1e-2  # guards f32r round-off pushing err^2 slightly negative

NIN = NJCH * 18 + NITILE * 6 + NJCH + NITILE  # 332 floats per partition


def _build(nc_holder=[], amp=None):
    if nc_holder and amp is None:
        return nc_holder[0]
    nc = bacc.Bacc(
        "TRN2",
        target_bir_lowering=False,
        debug=False,
        enable_asserts=True,
        num_devices=NCORES,
    )
    inp_in = nc.dram_tensor("inp", [128, NIN], F32, kind="ExternalInput").ap()
    out_dram = nc.dram_tensor("out", [ISLICE, N], F32, kind="ExternalOutput").ap()

    with tile.TileContext(nc) as tc, ExitStack() as ctx:
        if amp is None:
            _kernel_body(ctx, tc, out_dram, inp_in)
        else:
            with tc.For_i(0, amp, 1):
                with ExitStack() as ictx:
                    _kernel_body(ictx, tc, out_dram, inp_in)

    nc.compile()
    if amp is None:
        nc_holder.append(nc)
    return nc


U32 = mybir.dt.uint32
RSQRT_MAGIC = 0x5F3759DF


def _rsqrt_dve(nc, sb, eng, y, x, tag, iters=2):
    """y = rsqrt(x) computed entirely on one vector-ish engine: magic-
    constant seed (shift/xor/add on the bit pattern) + Newton iterations.
    Rel err <= ~5e-6 after 2 iterations -- plenty for basis normalization.
    Avoids the ACT sqrt round-trip on the critical path."""
    shape = list(y.shape)
    u = sb.tile(shape, U32, tag=f"rsu{tag}")
    eng.tensor_scalar(u[:], x.bitcast(U32), 1, None, OP.logical_shift_right)
    # y0 bits = MAGIC - (bits >> 1)  ==  (u xor 0xFFFFFFFF) + MAGIC + 1
    eng.tensor_scalar(
        y.bitcast(U32), u[:], 0xFFFFFFFF, RSQRT_MAGIC + 1, OP.bitwise_xor, OP.add
    )
    a = sb.tile(shape, F32, tag=f"rsa{tag}")
    for _ in range(iters):
        eng.tensor_mul(a[:], y, y)
        eng.tensor_mul(a[:], a[:], x)
        eng.tensor_scalar(a[:], a[:], -0.5, 1.5, OP.mult, OP.add)
        eng.tensor_mul(y, y, a[:])


def _frame_bases(nc, sb, eng, Fg, Est, g0, g1, fast=False):
    """Stage A for group range [g0, g1) on one engine; writes Est[:, g0:g1].
    fast=True keeps everything on `eng` via DVE-style rsqrt (no ACT hops);
    otherwise the two sqrt calls go to ACT and reciprocal runs on Vector."""
    P, C = 128, g1 - g0
    is_v = eng is nc.vector
    tg = f"{g0}"
    Fs = Fg[:, g0:g1]  # [p, g, pt, xyz]

    def red3(out, in_, tagbase):
        if is_v:
            nc.vector.reduce_sum(out.unsqueeze(3), in_, axis=mybir.AxisListType.X)
        else:
            t = sb.tile(list(out.shape), F32, tag=f"{tagbase}{tg}")
            eng.tensor_add(t[:], in_[:, :, :, 0], in_[:, :, :, 1])
            eng.tensor_add(out, t[:], in_[:, :, :, 2])

    w12 = sb.tile([P, C, 2, 3], F32, tag=f"w12{tg}")
    eng.tensor_sub(
        w12[:], Fs[:, :, 0::2, :], Fs[:, :, 1, :].unsqueeze(2).broadcast_to((P, C, 2, 3))
    )
    sqw = sb.tile([P, C, 2, 3], F32, tag=f"sqw{tg}")
    eng.tensor_mul(sqw[:], w12[:], w12[:])
    n2 = sb.tile([P, C, 2], F32, tag=f"n2{tg}")
    red3(n2[:], sqw[:], "rda")
    nrm = sb.tile([P, C, 2], F32, tag=f"nrm{tg}")
    if fast:
        ry = sb.tile([P, C, 2], F32, tag=f"ry{tg}")
        _rsqrt_dve(nc, sb, eng, ry[:], n2[:], f"a{tg}")
        eng.tensor_mul(nrm[:], n2[:], ry[:])  # |u| = n2 * rsqrt(n2)
    else:
        nc.scalar.sqrt(nrm[:], n2[:])

    # cross r = u x v via duplicated-copy rotation trick (no nrm dep:
    # scheduled while ACT does the sqrt)
    cbuf = sb.tile([P, C, 2, 6], F32, tag=f"cbuf{tg}")
    eng.tensor_copy(
        cbuf[:].rearrange("p g w (d x) -> p (g w) d x", d=2),
        w12[:]
        .rearrange("p g w x -> p (g w) x")
        .unsqueeze(2)
        .broadcast_to((P, C * 2, 2, 3)),
    )
    pqr = sb.tile([P, C, 3, 3], F32, tag=f"pqr{tg}")
    mtmp = sb.tile([P, C, 2, 3], F32, tag=f"mtmp{tg}")
    eng.tensor_mul(mtmp[:, :, 0, :], cbuf[:, :, 0, 1:4], cbuf[:, :, 1, 2:5])
    eng.tensor_mul(mtmp[:, :, 1, :], cbuf[:, :, 0, 2:5], cbuf[:, :, 1, 1:4])
    eng.tensor_sub(pqr[:, :, 2, :], mtmp[:, :, 0, :], mtmp[:, :, 1, :])

    tt = sb.tile([P, C, 2, 3], F32, tag=f"tt{tg}")
    eng.tensor_mul(
        tt[:, :, 0, :], w12[:, :, 0, :], nrm[:, :, 1].unsqueeze(2).broadcast_to((P, C, 3))
    )
    eng.tensor_mul(
        tt[:, :, 1, :], w12[:, :, 1, :], nrm[:, :, 0].unsqueeze(2).broadcast_to((P, C, 3))
    )
    eng.tensor_add(pqr[:, :, 0, :], tt[:, :, 0, :], tt[:, :, 1, :])
    eng.tensor_sub(pqr[:, :, 1, :], tt[:, :, 1, :], tt[:, :, 0, :])

    sq2 = sb.tile([P, C, 3, 3], F32, tag=f"sq2{tg}")
    eng.tensor_mul(
        sq2[:].rearrange("p g e x -> p (g e x)"),
        pqr[:].rearrange("p g e x -> p (g e x)"),
        pqr[:].rearrange("p g e x -> p (g e x)"),
    )
    n2b = sb.tile([P, C, 3], F32, tag=f"n2b{tg}")
    red3(n2b[:], sq2[:], "rdb")
    rinv = sb.tile([P, C, 3], F32, tag=f"rinv{tg}")
    if fast:
        _rsqrt_dve(nc, sb, eng, rinv[:], n2b[:], f"b{tg}")
    else:
        nrmb = sb.tile([P, C, 3], F32, tag=f"nrmb{tg}")
        nc.scalar.sqrt(nrmb[:], n2b[:])
        nc.vector.reciprocal(rinv[:], nrmb[:])
    eng.tensor_mul(
        Est[:, g0:g1], pqr[:], rinv[:].unsqueeze(3).broadcast_to((P, C, 3, 3))
    )


def _z_range(nc, sb, eng, Zb, Est, Ft, Mj, c0, c1):
    """Stage B (Z features) for j-chunks [c0, c1) on one engine."""
    P, C = 128, c1 - c0
    is_v = eng is nc.vector
    ch = slice(c0, c1)
    tg = f"{c0}"

    Estv = Est[:].rearrange("p (c s) e x -> p c s e x", s=2)
    Ep = Estv[:, ch, 0]  # [p, c, k, xyz]
    Et = Estv[:, ch, 1]
    bp = Ft[:, ch, 0, 1, :]  # [p, c, xyz]
    bt = Ft[:, ch, 1, 1, :]
    bpbt = Ft[:, ch, :, 1, :]  # [p, c, set, xyz]
    Zh = Zb[:, ch]
    Rz = Zh[:, :, 9:18].rearrange("p c (a b) -> p c a b", a=3)

    def red3(out, in_, tagbase):
        if is_v:
            nc.vector.reduce_sum(out.unsqueeze(3), in_, axis=mybir.AxisListType.X)
        else:
            t = sb.tile(list(out.shape), F32, tag=f"{tagbase}{tg}")
            eng.tensor_add(t[:], in_[:, :, :, 0], in_[:, :, :, 1])
            eng.tensor_add(out, t[:], in_[:, :, :, 2])

    # R[c,a,b] = sum_k Ep[c,k,a] * Et[c,k,b]  -> Zb slots 9..17
    prodR = sb.tile([P, C, 9, 3], F32, tag=f"prodR{tg}")
    for a in range(3):
        eng.tensor_mul(
            prodR[:, :, 3 * a : 3 * a + 3, :],
            Ep[:, :, :, a].unsqueeze(2).broadcast_to((P, C, 3, 3)),
            Et.transpose([0, 1, 3, 2]),
        )
    red3(Zh[:, :, 9:18], prodR[:], "rdr")

    # Rbt[c,a] = sum_b R[c,a,b] bt[c,b] ; Rtbp[c,b] = sum_a R[c,a,b] bp[c,a]
    prodv = sb.tile([P, C, 6, 3], F32, tag=f"prodv{tg}")
    eng.tensor_mul(prodv[:, :, 0:3, :], Rz, bt.unsqueeze(2).broadcast_to((P, C, 3, 3)))
    eng.tensor_mul(
        prodv[:, :, 3:6, :],
        Rz.transpose([0, 1, 3, 2]),
        bp.unsqueeze(2).broadcast_to((P, C, 3, 3)),
    )
    Rv = sb.tile([P, C, 2, 3], F32, tag=f"Rv{tg}")
    red3(Rv[:].rearrange("p c v x -> p c (v x)"), prodv[:], "rdv")

    # Zb[3:9] = (bp, bt) - (Rbt, Rtbp)
    eng.tensor_sub(
        Zh[:, :, 3:9].rearrange("p c (s x) -> p c s x", s=2), bpbt, Rv[:]
    )
    # z0 = bp.(bp - 2 Rbt) + bt.bt ; (bp - 2 Rbt) = Zb[3:6] - Rbt
    ph = sb.tile([P, C, 2, 3], F32, tag=f"ph{tg}")
    eng.tensor_sub(ph[:, :, 0, :], Zh[:, :, 3:6], Rv[:, :, 0, :])
    eng.tensor_copy(ph[:, :, 1, :], bt)
    prodH = sb.tile([P, C, 2, 3], F32, tag=f"prodH{tg}")
    eng.tensor_mul(prodH[:], bpbt, ph[:])
    if is_v:
        nc.vector.reduce_sum(
            Zh[:, :, 0:1],
            prodH[:].rearrange("p c s x -> p c (s x)"),
            axis=mybir.AxisListType.X,
        )
    else:
        t = sb.tile([P, C, 3], F32, tag=f"rdz{tg}")
        eng.tensor_add(t[:], prodH[:, :, 0, :], prodH[:, :, 1, :])
        t2 = sb.tile([P, C, 1], F32, tag=f"rdz2{tg}")
        eng.tensor_add(t2[:], t[:, :, 0:1], t[:, :, 1:2])
        eng.tensor_add(Zh[:, :, 0:1], t2[:], t[:, :, 2:3])

    eng.memset(Zh[:, :, 1:3], 1.0)
    eng.tensor_mul(
        Zh[:, :, 0:NF],
        Zh[:, :, 0:NF],
        Mj[:, ch].unsqueeze(2).broadcast_to((P, C, NF)),
    )


def _kernel_body(ctx, tc, out_dram, inp_in):
    nc = tc.nc
    P = 128
    sb = ctx.enter_context(tc.tile_pool(name="sb", bufs=1))
    outp = ctx.enter_context(tc.tile_pool(name="outp", bufs=8))
    pst = ctx.enter_context(tc.tile_pool(name="pst", bufs=2, space="PSUM"))
    ps5 = ctx.enter_context(tc.tile_pool(name="ps5", bufs=4, space="PSUM"))

    # ---- one merged input DMA --------------------------------------------
    inp = sb.tile([P, NIN], F32, tag="inp")
    nc.sync.dma_start(out=inp[:], in_=inp_in[:])
    o0 = NJCH * 18
    o1 = o0 + NITILE * 6
    o2 = o1 + NJCH
    # frames chunk-major: [p, chunk, set, pt, xyz]
    Ft = inp[:, 0:o0].rearrange("p (c s t x) -> p c s t x", c=NJCH, s=2, t=3)
    Ct = inp[:, o0:o1].rearrange("p (c s x) -> p c s x", c=NITILE, s=2)
    Mj = inp[:, o1:o2]  # [p, NJCH]
    Mi = inp[:, o2:NIN]  # [p, NITILE]
    Fg = Ft[:].rearrange("p c s t x -> p (c s) t x")  # [p, g, pt, xyz]

    ident = sb.tile([P, P], F32, tag="ident")
    make_identity(nc, ident[:])
    bias_t = sb.tile([P, 1], F32, tag="bias")
    nc.gpsimd.memset(bias_t[:], SQRT_BIAS)
    # dependency-free sqrt pins the sqrt_and_others ACT table at t=0
    dumm = sb.tile([P, 1], F32, tag="dumm")
    nc.scalar.sqrt(dumm[:], bias_t[:])

    # ---- Y features (Pool; only needs coords) -----------------------------
    Yb = sb.tile([P, NITILE, FPAD], F32, tag="Yb")
    nc.gpsimd.tensor_scalar_mul(
        Yb[:, :, 3:9], Ct[:].rearrange("p c s x -> p c (s x)"), -2.0
    )
    sqc = sb.tile([P, NITILE, 2, 3], F32, tag="sqc")
    nc.gpsimd.tensor_mul(
        sqc[:].rearrange("p c s x -> p (c s x)"),
        Ct[:].rearrange("p c s x -> p (c s x)"),
        Ct[:].rearrange("p c s x -> p (c s x)"),
    )
    # |pc|^2, |tc|^2 via strided adds (Pool can't free-reduce)
    yt1 = sb.tile([P, NITILE, 2], F32, tag="yt1")
    nc.gpsimd.tensor_add(yt1[:], sqc[:, :, :, 0], sqc[:, :, :, 1])
    nc.gpsimd.tensor_add(Yb[:, :, 1:3], yt1[:], sqc[:, :, :, 2])
    # Yb[9:18] = (-2 pc) (x) tc
    nc.gpsimd.tensor_mul(
        Yb[:, :, 9:18].rearrange("p c (a b) -> p c a b", a=3),
        Yb[:, :, 3:6].unsqueeze(3).broadcast_to((P, NITILE, 3, 3)),
        Ct[:, :, 1, :].unsqueeze(2).broadcast_to((P, NITILE, 3, 3)),
    )
    nc.gpsimd.memset(Yb[:, :, 0:1], 1.0)
    nc.gpsimd.tensor_mul(
        Yb[:, :, 0:NF],
        Yb[:, :, 0:NF],
        Mi[:].unsqueeze(2).broadcast_to((P, NITILE, NF)),
    )

    # ---- stage A + B + transposes + main loop, emitted in pipeline order --
    mm_dt = F32R if USE_F32R else F32
    Est = sb.tile([P, G, 3, 3], F32, tag="Est")
    Zb = sb.tile([P, NJCH, FPAD], F32, tag="Zb")
    YT = sb.tile([NF, NITILE * P], mm_dt, tag="YT")
    ZT = sb.tile([NF, N], mm_dt, tag="ZT")

    def z_transpose(g, engs):
        """Transpose+copy Zb chunks 4g..4g+3 into ZT."""
        ptz = pst.tile([P, P], F32, tag="tp")
        nc.tensor.transpose(
            ptz[:], Zb[:, 4 * g : 4 * g + 4, :].rearrange("p c f -> p (c f)"), ident[:]
        )
        for cl in range(4):
            c = 4 * g + cl
            dst = ZT[0:NF, c * P : (c + 1) * P]
            src = ptz[cl * FPAD : cl * FPAD + NF, :]
            eng = engs[cl]
            (eng.copy if eng is nc.scalar else eng.tensor_copy)(dst, src)

    def block(it, col0):
        pm = ps5.tile([P, 512], F32, tag="mm")
        nc.tensor.matmul(
            pm[:],
            YT[0:NF, it * P : (it + 1) * P],
            ZT[0:NF, col0 : col0 + 512],
            start=True,
            stop=True,
        )
        ot = outp.tile([P, 512], F32, tag="ot")
        nc.scalar.activation(ot[:], pm[:], AF.Sqrt, bias=bias_t[:])
        nc.sync.dma_start(
            out=out_dram[it * P : (it + 1) * P, col0 : col0 + 512], in_=ot[:]
        )

    # fast path: chunks 0-3 end-to-end feeds the first four output blocks
    with tc.high_priority():
        _frame_bases(nc, sb, nc.vector, Fg, Est, 0, 8)  # chunks 0-3
        _z_range(nc, sb, nc.vector, Zb[:], Est, Ft, Mj, 0, 4)

    _frame_bases(nc, sb, nc.vector, Fg, Est, 8, G)  # chunks 4-15

    z_transpose(0, [nc.vector] * 4)
    # Y transpose + copies AFTER the g0 transpose: the Y copies land on
    # ACT behind the fast-path sqrts instead of ahead of them (head-of-
    # line), and are still ready before the first matmul needs YT
    pt = pst.tile([P, P], F32, tag="tp")
    nc.tensor.transpose(pt[:], Yb[:].rearrange("p c f -> p (c f)"), ident[:])
    for c in range(NITILE):
        nc.scalar.copy(YT[0:NF, c * P : (c + 1) * P], pt[c * FPAD : c * FPAD + NF, :])
    for it in range(NITILE):
        block(it, 0)

    _z_range(nc, sb, nc.vector, Zb[:], Est, Ft, Mj, 4, 8)
    z_transpose(1, [nc.vector] * 4)
    for it in range(NITILE):
        block(it, 512)

    _z_range(nc, sb, nc.gpsimd, Zb[:], Est, Ft, Mj, 8, 16)
    z_transpose(2, [nc.vector] * 4)
    for it in range(NITILE):
        block(it, 1024)
    z_transpose(3, [nc.vector] * 4)
    for it in range(NITILE):
        block(it, 1536)


def _shard_inputs(pred_coords, true_coords, pred_frames, true_frames, mask):
    """Host-side reformat into per-core DMA-friendly layouts."""
    pc = np.asarray(pred_coords, np.float32)
    tc = np.asarray(true_coords, np.float32)
    pf = np.asarray(pred_frames, np.float32)
    tf = np.asarray(true_frames, np.float32)
    mk = np.asarray(mask).astype(np.float32)

    in_maps = []
    for core in range(NCORES):
        b = core // (NCORES // B)
        i0 = (core % (NCORES // B)) * ISLICE
        # frames [128, chunk, set, pt, xyz] ; input frames are [n, xyz, pt]
        fr = np.stack([pf[b], tf[b]], axis=0)  # [2, n, 3xyz, 3pt]
        fr = fr.transpose(0, 1, 3, 2)  # [2, n, pt, xyz]
        fr = fr.reshape(2, NJCH, 128, 3, 3).transpose(2, 1, 0, 3, 4)
        frames = fr.reshape(128, -1)
        # coords [128, chunk, set, xyz]
        co = np.stack([pc[b, i0 : i0 + ISLICE], tc[b, i0 : i0 + ISLICE]], axis=1)
        co = co.reshape(NITILE, 128, 2, 3).transpose(1, 0, 2, 3)
        coords = co.reshape(128, -1)
        maskj = mk[b].reshape(NJCH, 128).T
        maski = mk[b, i0 : i0 + ISLICE].reshape(NITILE, 128).T
        inp = np.ascontiguousarray(
            np.concatenate([frames, coords, maskj, maski], axis=1)
        )
        in_maps.append({"inp": inp})
    return in_maps


def kernel(pred_coords, true_coords, pred_frames, true_frames, mask, _res=[]):
    nc = _build()
    in_maps = _shard_inputs(pred_coords, true_coords, pred_frames, true_frames, mask)
    res = run_bass_kernel_spmd(nc, in_maps, list(range(NCORES)))
    _res.clear()
    _res.append(res)
    out = np.empty((B, N, N), np.float32)
    for core in range(NCORES):
        b = core // (NCORES // B)
        i0 = (core % (NCORES // B)) * ISLICE
        out[b, i0 : i0 + ISLICE, :] = res.results[core]["out"]
    return out


if __name__ == "__main__":
    rng = np.random.default_rng(0)
    ins = {
        "pred_coords": rng.standard_normal((B, N, 3)).astype(np.float32),
        "true_coords": rng.standard_normal((B, N, 3)).astype(np.float32),
        "pred_frames": rng.standard_normal((B, N, 3, 3)).astype(np.float32),
        "true_frames": rng.standard_normal((B, N, 3, 3)).astype(np.float32),
        "mask": np.ones((B, N), bool),
    }
    out = kernel(**ins)
    print("out", out.shape, out.dtype, float(np.abs(out).max()))


# revision 55
# speedup vs baseline: 5.7831x; 3.2918x over previous
"""ComputeAlignmentError kernel for 8 TRN2 NeuronCores.

Math: for each batch b, pairwise alignment error
    err[i,j] = || Ep_j (pc_i - bp_j) - Et_j (tc_i - bt_j) + eps ||_2
where Ep/Et are orthonormal frame bases built from pred/true frames and
bp/bt are the frame origins.  Because Ep/Et are rotations, err^2 collapses
into a rank-18 bilinear form  err^2[i,j] = Y[i] . Z[j]  with
    Y[i] = [1, |pc|^2, |tc|^2, -2pc, -2tc, -2vec(pc tc^T)]   (18)
    Z[j] = [z0, 1, 1, (bp - R bt), (bt - R^T bp), vec(R)]
    R_j = Ep_j^T Et_j,  z0 = bp.(bp - 2 R bt) + bt.bt
(the reference's eps=1e-8 terms shift err^2 by ~1e-7 -- far below fp32
noise -- and are dropped).  Mask folds in: Y *= mask_i, Z *= mask_j.

Frame basis without normalizing w1/w2 first (saves a normalize round):
    u = a - b, v = c - b
    e1 = normalize(|v| u + |u| v), e2 = normalize(|u| v - |v| u),
    e3 = normalize(u x v)        (e1 _|_ e2 exactly, so |e1 x e2| = 1)

Each core handles one (batch, 512-row i-slice).  Frames are laid out
chunk-major (chunk, set) so a leading slice of chunks for BOTH sets is
one contiguous group range.  Critical-path ordering:
  - one merged input DMA; dummy sqrt pins the ACT sqrt table at t=0
  - stage A for chunks 0-3 first (Vector), then chunks 4-15
  - stage B: chunks 0-3, 4-7 on Vector; 8-15 on Pool; Y on Pool
  - PE transposes; PSUM->SBUF copies spread over ACT/Vector/Pool
  - output column blocks ordered j-low-first across all i-tiles so the
    out-DMA stream starts as early as possible; K=18 float32r matmuls
    (4x PE rate, no row packing); sqrt+bias on ACT guards f32r round-off
"""

import sys

import numpy as np

sys.path.insert(0, "/opt/trn_rl_repo")

from contextlib import ExitStack

import concourse.bacc as bacc
import concourse.bass as bass
import concourse.tile as tile
from concourse import mybir
from concourse.bass_utils import run_bass_kernel_spmd
from concourse.masks import make_identity

F32 = mybir.dt.float32
F32R = mybir.dt.float32r
AF = mybir.ActivationFunctionType
OP = mybir.AluOpType

B, N = 2, 2048
NCORES = 8
ISLICE = N * B // NCORES  # 512 rows of i per core
NITILE = ISLICE // 128  # 4 i-tiles per core
NJCH = N // 128  # 16 j-chunks
NF = 18  # feature count K
FPAD = 32  # feature slot padding (PSUM partition alignment after transpose)
G = 2 * NJCH  # 32 (chunk, set) groups

USE_F32R = True
SQRT_BIAS = 1e-2  # guards f32r round-off pushing err^2 slightly negative

NIN = NJCH * 18 + NITILE * 6 + NJCH + NITILE  # 332 floats per partition


def _build(nc_holder=[], amp=None):
    if nc_holder and amp is None:
        return nc_holder[0]
    nc = bacc.Bacc(
        "TRN2",
        target_bir_lowering=False,
        debug=False,
        enable_asserts=True,
        num_devices=NCORES,
    )
    inp_in = nc.dram_tensor("inp", [128, NIN], F32, kind="ExternalInput").ap()
    out_dram = nc.dram_tensor("out", [ISLICE, N], F32, kind="ExternalOutput").ap()

    with tile.TileContext(nc) as tc, ExitStack() as ctx:
        if amp is None:
            _kernel_body(ctx, tc, out_dram, inp_in)
        else:
            with tc.For_i(0, amp, 1):
                with ExitStack() as ictx:
                    _kernel_body(ictx, tc, out_dram, inp_in)

    nc.compile()
    if amp is None:
        nc_holder.append(nc)
    return nc


U32 = mybir.dt.uint32
RSQRT_MAGIC = 0x5F3759DF


def _rsqrt_dve(nc, sb, eng, y, x, tag, iters=2):
    """y = rsqrt(x) computed entirely on one vector-ish engine: magic-
    constant seed (shift/xor/add on the bit pattern) + Newton iterations.
    Rel err <= ~5e-6 after 2 iterations -- plenty for basis normalization.
    Avoids the ACT sqrt round-trip on the critical path."""
    shape = list(y.shape)
    u = sb.tile(shape, U32, tag=f"rsu{tag}")
    eng.tensor_scalar(u[:], x.bitcast(U32), 1, None, OP.logical_shift_right)
    # y0 bits = MAGIC - (bits >> 1)  ==  (u xor 0xFFFFFFFF) + MAGIC + 1
    eng.tensor_scalar(
        y.bitcast(U32), u[:], 0xFFFFFFFF, RSQRT_MAGIC + 1, OP.bitwise_xor, OP.add
    )
    a = sb.tile(shape, F32, tag=f"rsa{tag}")
    for _ in range(iters):
        eng.tensor_mul(a[:], y, y)
        eng.tensor_mul(a[:], a[:], x)
        eng.tensor_scalar(a[:], a[:], -0.5, 1.5, OP.mult, OP.add)
        eng.tensor_mul(y, y, a[:])


def _frame_bases(nc, sb, eng, Fg, Est, g0, g1, fast=False):
    """Stage A for group range [g0, g1) on one engine; writes Est[:, g0:g1].
    fast=True keeps everything on `eng` via DVE-style rsqrt (no ACT hops);
    otherwise the two sqrt calls go to ACT and reciprocal runs on Vector."""
    P, C = 128, g1 - g0
    is_v = eng is nc.vector
    tg = f"{g0}"
    Fs = Fg[:, g0:g1]  # [p, g, pt, xyz]

    def red3(out, in_, tagbase):
        if is_v:
            nc.vector.reduce_sum(out.unsqueeze(3), in_, axis=mybir.AxisListType.X)
        else:
            t = sb.tile(list(out.shape), F32, tag=f"{tagbase}{tg}")
            eng.tensor_add(t[:], in_[:, :, :, 0], in_[:, :, :, 1])
            eng.tensor_add(out, t[:], in_[:, :, :, 2])

    w12 = sb.tile([P, C, 2, 3], F32, tag=f"w12{tg}")
    eng.tensor_sub(
        w12[:], Fs[:, :, 0::2, :], Fs[:, :, 1, :].unsqueeze(2).broadcast_to((P, C, 2, 3))
    )
    sqw = sb.tile([P, C, 2, 3], F32, tag=f"sqw{tg}")
    eng.tensor_mul(sqw[:], w12[:], w12[:])
    n2 = sb.tile([P, C, 2], F32, tag=f"n2{tg}")
    red3(n2[:], sqw[:], "rda")
    nrm = sb.tile([P, C, 2], F32, tag=f"nrm{tg}")
    if fast:
        ry = sb.tile([P, C, 2], F32, tag=f"ry{tg}")
        _rsqrt_dve(nc, sb, eng, ry[:], n2[:], f"a{tg}")
        eng.tensor_mul(nrm[:], n2[:], ry[:])  # |u| = n2 * rsqrt(n2)
    else:
        nc.scalar.sqrt(nrm[:], n2[:])

    # cross r = u x v via duplicated-copy rotation trick (no nrm dep:
    # scheduled while ACT does the sqrt)
    cbuf = sb.tile([P, C, 2, 6], F32, tag=f"cbuf{tg}")
    eng.tensor_copy(
        cbuf[:].rearrange("p g w (d x) -> p (g w) d x", d=2),
        w12[:]
        .rearrange("p g w x -> p (g w) x")
        .unsqueeze(2)
        .broadcast_to((P, C * 2, 2, 3)),
    )
    pqr = sb.tile([P, C, 3, 3], F32, tag=f"pqr{tg}")
    mtmp = sb.tile([P, C, 2, 3], F32, tag=f"mtmp{tg}")
    eng.tensor_mul(mtmp[:, :, 0, :], cbuf[:, :, 0, 1:4], cbuf[:, :, 1, 2:5])
    eng.tensor_mul(mtmp[:, :, 1, :], cbuf[:, :, 0, 2:5], cbuf[:, :, 1, 1:4])
    eng.tensor_sub(pqr[:, :, 2, :], mtmp[:, :, 0, :], mtmp[:, :, 1, :])

    tt = sb.tile([P, C, 2, 3], F32, tag=f"tt{tg}")
    eng.tensor_mul(
        tt[:, :, 0, :], w12[:, :, 0, :], nrm[:, :, 1].unsqueeze(2).broadcast_to((P, C, 3))
    )
    eng.tensor_mul(
        tt[:, :, 1, :], w12[:, :, 1, :], nrm[:, :, 0].unsqueeze(2).broadcast_to((P, C, 3))
    )
    eng.tensor_add(pqr[:, :, 0, :], tt[:, :, 0, :], tt[:, :, 1, :])
    eng.tensor_sub(pqr[:, :, 1, :], tt[:, :, 1, :], tt[:, :, 0, :])

    sq2 = sb.tile([P, C, 3, 3], F32, tag=f"sq2{tg}")
    eng.tensor_mul(
        sq2[:].rearrange("p g e x -> p (g e x)"),
        pqr[:].rearrange("p g e x -> p (g e x)"),
        pqr[:].rearrange("p g e x -> p (g e x)"),
    )
    n2b = sb.tile([P, C, 3], F32, tag=f"n2b{tg}")
    red3(n2b[:], sq2[:], "rdb")
    rinv = sb.tile([P, C, 3], F32, tag=f"rinv{tg}")
    if fast:
        _rsqrt_dve(nc, sb, eng, rinv[:], n2b[:], f"b{tg}")
    else:
        nrmb = sb.tile([P, C, 3], F32, tag=f"nrmb{tg}")
        nc.scalar.sqrt(nrmb[:], n2b[:])
        nc.vector.reciprocal(rinv[:], nrmb[:])
    eng.tensor_mul(
        Est[:, g0:g1], pqr[:], rinv[:].unsqueeze(3).broadcast_to((P, C, 3, 3))
    )


def _z_range(nc, sb, eng, Zb, Est, Ft, Mj, c0, c1):
    """Stage B (Z features) for j-chunks [c0, c1) on one engine."""
    P, C = 128, c1 - c0
    is_v = eng is nc.vector
    ch = slice(c0, c1)
    tg = f"{c0}"

    Estv = Est[:].rearrange("p (c s) e x -> p c s e x", s=2)
    Ep = Estv[:, ch, 0]  # [p, c, k, xyz]
    Et = Estv[:, ch, 1]
    bp = Ft[:, ch, 0, 1, :]  # [p, c, xyz]
    bt = Ft[:, ch, 1, 1, :]
    bpbt = Ft[:, ch, :, 1, :]  # [p, c, set, xyz]
    Zh = Zb[:, ch]
    Rz = Zh[:, :, 9:18].rearrange("p c (a b) -> p c a b", a=3)

    def red3(out, in_, tagbase):
        if is_v:
            nc.vector.reduce_sum(out.unsqueeze(3), in_, axis=mybir.AxisListType.X)
        else:
            t = sb.tile(list(out.shape), F32, tag=f"{tagbase}{tg}")
            eng.tensor_add(t[:], in_[:, :, :, 0], in_[:, :, :, 1])
            eng.tensor_add(out, t[:], in_[:, :, :, 2])

    # R[c,a,b] = sum_k Ep[c,k,a] * Et[c,k,b]  -> Zb slots 9..17
    prodR = sb.tile([P, C, 9, 3], F32, tag=f"prodR{tg}")
    for a in range(3):
        eng.tensor_mul(
            prodR[:, :, 3 * a : 3 * a + 3, :],
            Ep[:, :, :, a].unsqueeze(2).broadcast_to((P, C, 3, 3)),
            Et.transpose([0, 1, 3, 2]),
        )
    red3(Zh[:, :, 9:18], prodR[:], "rdr")

    # Rbt[c,a] = sum_b R[c,a,b] bt[c,b] ; Rtbp[c,b] = sum_a R[c,a,b] bp[c,a]
    prodv = sb.tile([P, C, 6, 3], F32, tag=f"prodv{tg}")
    eng.tensor_mul(prodv[:, :, 0:3, :], Rz, bt.unsqueeze(2).broadcast_to((P, C, 3, 3)))
    eng.tensor_mul(
        prodv[:, :, 3:6, :],
        Rz.transpose([0, 1, 3, 2]),
        bp.unsqueeze(2).broadcast_to((P, C, 3, 3)),
    )
    Rv = sb.tile([P, C, 2, 3], F32, tag=f"Rv{tg}")
    red3(Rv[:].rearrange("p c v x -> p c (v x)"), prodv[:], "rdv")

    # Zb[3:9] = (bp, bt) - (Rbt, Rtbp)
    eng.tensor_sub(
        Zh[:, :, 3:9].rearrange("p c (s x) -> p c s x", s=2), bpbt, Rv[:]
    )
    # z0 = bp.(bp - 2 Rbt) + bt.bt ; (bp - 2 Rbt) = Zb[3:6] - Rbt
    ph = sb.tile([P, C, 2, 3], F32, tag=f"ph{tg}")
    eng.tensor_sub(ph[:, :, 0, :], Zh[:, :, 3:6], Rv[:, :, 0, :])
    eng.tensor_copy(ph[:, :, 1, :], bt)
    prodH = sb.tile([P, C, 2, 3], F32, tag=f"prodH{tg}")
    eng.tensor_mul(prodH[:], bpbt, ph[:])
    if is_v:
        nc.vector.reduce_sum(
            Zh[:, :, 0:1],
            prodH[:].rearrange("p c s x -> p c (s x)"),
            axis=mybir.AxisListType.X,
        )
    else:
        t = sb.tile([P, C, 3], F32, tag=f"rdz{tg}")
        eng.tensor_add(t[:], prodH[:, :, 0, :], prodH[:, :, 1, :])
        t2 = sb.tile([P, C, 1], F32, tag=f"rdz2{tg}")
        eng.tensor_add(t2[:], t[:, :, 0:1], t[:, :, 1:2])
        eng.tensor_add(Zh[:, :, 0:1], t2[:], t[:, :, 2:3])

    eng.memset(Zh[:, :, 1:3], 1.0)
    eng.tensor_mul(
        Zh[:, :, 0:NF],
        Zh[:, :, 0:NF],
        Mj[:, ch].unsqueeze(2).broadcast_to((P, C, NF)),
    )


def _kernel_body(ctx, tc, out_dram, inp_in):
    nc = tc.nc
    P = 128
    sb = ctx.enter_context(tc.tile_pool(name="sb", bufs=1))
    outp = ctx.enter_context(tc.tile_pool(name="outp", bufs=8))
    pst = ctx.enter_context(tc.tile_pool(name="pst", bufs=2, space="PSUM"))
    ps5 = ctx.enter_context(tc.tile_pool(name="ps5", bufs=4, space="PSUM"))

    # ---- one merged input DMA --------------------------------------------
    inp = sb.tile([P, NIN], F32, tag="inp")
    nc.sync.dma_start(out=inp[:], in_=inp_in[:])
    o0 = NJCH * 18
    o1 = o0 + NITILE * 6
    o2 = o1 + NJCH
    # frames chunk-major: [p, chunk, set, pt, xyz]
    Ft = inp[:, 0:o0].rearrange("p (c s t x) -> p c s t x", c=NJCH, s=2, t=3)
    Ct = inp[:, o0:o1].rearrange("p (c s x) -> p c s x", c=NITILE, s=2)
    Mj = inp[:, o1:o2]  # [p, NJCH]
    Mi = inp[:, o2:NIN]  # [p, NITILE]
    Fg = Ft[:].rearrange("p c s t x -> p (c s) t x")  # [p, g, pt, xyz]

    ident = sb.tile([P, P], F32, tag="ident")
    make_identity(nc, ident[:])
    bias_t = sb.tile([P, 1], F32, tag="bias")
    nc.gpsimd.memset(bias_t[:], SQRT_BIAS)
    # dependency-free sqrt pins the sqrt_and_others ACT table at t=0
    dumm = sb.tile([P, 1], F32, tag="dumm")
    nc.scalar.sqrt(dumm[:], bias_t[:])

    # ---- Y features (Pool; only needs coords) -----------------------------
    Yb = sb.tile([P, NITILE, FPAD], F32, tag="Yb")
    nc.gpsimd.tensor_scalar_mul(
        Yb[:, :, 3:9], Ct[:].rearrange("p c s x -> p c (s x)"), -2.0
    )
    sqc = sb.tile([P, NITILE, 2, 3], F32, tag="sqc")
    nc.gpsimd.tensor_mul(
        sqc[:].rearrange("p c s x -> p (c s x)"),
        Ct[:].rearrange("p c s x -> p (c s x)"),
        Ct[:].rearrange("p c s x -> p (c s x)"),
    )
    # |pc|^2, |tc|^2 via strided adds (Pool can't free-reduce)
    yt1 = sb.tile([P, NITILE, 2], F32, tag="yt1")
    nc.gpsimd.tensor_add(yt1[:], sqc[:, :, :, 0], sqc[:, :, :, 1])
    nc.gpsimd.tensor_add(Yb[:, :, 1:3], yt1[:], sqc[:, :, :, 2])
    # Yb[9:18] = (-2 pc) (x) tc
    nc.gpsimd.tensor_mul(
        Yb[:, :, 9:18].rearrange("p c (a b) -> p c a b", a=3),
        Yb[:, :, 3:6].unsqueeze(3).broadcast_to((P, NITILE, 3, 3)),
        Ct[:, :, 1, :].unsqueeze(2).broadcast_to((P, NITILE, 3, 3)),
    )
    nc.gpsimd.memset(Yb[:, :, 0:1], 1.0)
    nc.gpsimd.tensor_mul(
        Yb[:, :, 0:NF],
        Yb[:, :, 0:NF],
        Mi[:].unsqueeze(2).broadcast_to((P, NITILE, NF)),
    )

    # ---- stage A + B + transposes + main loop, emitted in pipeline order --
    mm_dt = F32R if USE_F32R else F32
    Est = sb.tile([P, G, 3, 3], F32, tag="Est")
    Zb = sb.tile([P, NJCH, FPAD], F32, tag="Zb")
    YT = sb.tile([NF, NITILE * P], mm_dt, tag="YT")
    ZT = sb.tile([NF, N], mm_dt, tag="ZT")

    def z_transpose(g, engs):
        """Transpose+copy Zb chunks 4g..4g+3 into ZT."""
        ptz = pst.tile([P, P], F32, tag="tp")
        nc.tensor.transpose(
            ptz[:], Zb[:, 4 * g : 4 * g + 4, :].rearrange("p c f -> p (c f)"), ident[:]
        )
        for cl in range(4):
            c = 4 * g + cl
            dst = ZT[0:NF, c * P : (c + 1) * P]
            src = ptz[cl * FPAD : cl * FPAD + NF, :]
            eng = engs[cl]
            (eng.copy if eng is nc.scalar else eng.tensor_copy)(dst, src)

    def block(it, col0):
        pm = ps5.tile([P, 512], F32, tag="mm")
        nc.tensor.matmul(
            pm[:],
            YT[0:NF, it * P : (it + 1) * P],
            ZT[0:NF, col0 : col0 + 512],
            start=True,
            stop=True,
        )
        ot = outp.tile([P, 512], F32, tag="ot")
        nc.scalar.activation(ot[:], pm[:], AF.Sqrt, bias=bias_t[:])
        nc.sync.dma_start(
            out=out_dram[it * P : (it + 1) * P, col0 : col0 + 512], in_=ot[:]
        )

    # fast path: chunks 0-3 end-to-end feeds the first four output blocks
    with tc.high_priority():
        _frame_bases(nc, sb, nc.vector, Fg, Est, 0, 8)  # chunks 0-3
        _z_range(nc, sb, nc.vector, Zb[:], Est, Ft, Mj, 0, 4)

    _frame_bases(nc, sb, nc.vector, Fg, Est, 8, G)  # chunks 4-15

    z_transpose(0, [nc.vector] * 4)
    # Y transpose + copies AFTER the g0 transpose: the Y copies land on
    # ACT behind the fast-path sqrts instead of ahead of them (head-of-
    # line), and are still ready before the first matmul needs YT
    pt = pst.tile([P, P], F32, tag="tp")
    nc.tensor.transpose(pt[:], Yb[:].rearrange("p c f -> p (c f)"), ident[:])
    for c in range(NITILE):
        nc.scalar.copy(YT[0:NF, c * P : (c + 1) * P], pt[c * FPAD : c * FPAD + NF, :])
    for it in range(NITILE):
        block(it, 0)

    _z_range(nc, sb, nc.vector, Zb[:], Est, Ft, Mj, 4, 8)
    z_transpose(1, [nc.vector] * 4)
    for it in range(NITILE):
        block(it, 512)

    _z_range(nc, sb, nc.gpsimd, Zb[:], Est, Ft, Mj, 8, 16)
    z_transpose(2, [nc.vector] * 4)
    for it in range(NITILE):
        block(it, 1024)
    z_transpose(3, [nc.vector] * 4)
    for it in range(NITILE):
        block(it, 1536)


def _shard_inputs(pred_coords, true_coords, pred_frames, true_frames, mask):
    """Host-side reformat into per-core DMA-friendly layouts."""
    pc = np.asarray(pred_coords, np.float32)
    tc = np.asarray(true_coords, np.float32)
    pf = np.asarray(pred_frames, np.float32)
    tf = np.asarray(true_frames, np.float32)
    mk = np.asarray(mask).astype(np.float32)

    in_maps = []
    for core in range(NCORES):
        b = core // (NCORES // B)
        i0 = (core % (NCORES // B)) * ISLICE
        # frames [128, chunk, set, pt, xyz] ; input frames are [n, xyz, pt]
        fr = np.stack([pf[b], tf[b]], axis=0)  # [2, n, 3xyz, 3pt]
        fr = fr.transpose(0, 1, 3, 2)  # [2, n, pt, xyz]
        fr = fr.reshape(2, NJCH, 128, 3, 3).transpose(2, 1, 0, 3, 4)
        frames = fr.reshape(128, -1)
        # coords [128, chunk, set, xyz]
        co = np.stack([pc[b, i0 : i0 + ISLICE], tc[b, i0 : i0 + ISLICE]], axis=1)
        co = co.reshape(NITILE, 128, 2, 3).transpose(1, 0, 2, 3)
        coords = co.reshape(128, -1)
        maskj = mk[b].reshape(NJCH, 128).T
        maski = mk[b, i0 : i0 + ISLICE].reshape(NITILE, 128).T
        inp = np.ascontiguousarray(
            np.concatenate([frames, coords, maskj, maski], axis=1)
        )
        in_maps.append({"inp": inp})
    return in_maps


def kernel(pred_coords, true_coords, pred_frames, true_frames, mask, _res=[]):
    nc = _build()
    in_maps = _shard_inputs(pred_coords, true_coords, pred_frames, true_frames, mask)
    res = run_bass_kernel_spmd(nc, in_maps, list(range(NCORES)))
    _res.clear()
    _res.append(res)
    out = np.empty((B, N, N), np.float32)
    for core in range(NCORES):
        b = core // (NCORES // B)
        i0 = (core % (NCORES // B)) * ISLICE
        out[b, i0 : i0 + ISLICE, :] = res.results[core]["out"]
    return out


if __name__ == "__main__":
    rng = np.random.default_rng(0)
    ins = {
        "pred_coords": rng.standard_normal((B, N, 3)).astype(np.float32),
        "true_coords": rng.standard_normal((B, N, 3)).astype(np.float32),
        "pred_frames": rng.standard_normal((B, N, 3, 3)).astype(np.float32),
        "true_frames": rng.standard_normal((B, N, 3, 3)).astype(np.float32),
        "mask": np.ones((B, N), bool),
    }
    out = kernel(**ins)
    print("out", out.shape, out.dtype, float(np.abs(out).max()))
